# revision 15
# baseline (speedup 1.0000x reference)
"""Multi-head attention (B=4, S=2048, d_model=1024, h=16) on 8 TRN2 NeuronCores.

Sharding: data-parallel over batch (4) x tensor-parallel over head-groups (2 x 8
heads, column-split Wq/Wk/Wv, row-split Wo). Each core computes a full (2048,
1024) partial of the output projection for its (batch, head-group); the host
sums the two group partials per batch and adds bo.

Device kernel (identical SPMD program on all 8 cores):
  qT/kT = W @ X.T computed directly in head-major layout (TF32 matmuls, PE at
  full rate), scoresT = k @ qT per head with 64x128 row-tiled matmul pairs (two
  heads run concurrently on the two halves of the PE array), one 2048-wide exp
  per 4-bank PSUM block on the scalar engine (the per-instruction overhead
  makes narrow activations 40% slower), AV as [v|1].T @ exps so the softmax
  denominators fall out of the matmul for free, normalization via vector-engine
  reciprocal + gpsimd partition-broadcast, then the output projection from the
  already-transposed attention output.
"""
import numpy as np

import concourse.bacc as bacc
import concourse.mybir as mybir
from concourse.tile import TileContext
from concourse.bass_utils import run_bass_kernel_spmd

P = 128
S = 2048          # sequence length
DM = 1024         # d_model
DG = 512          # dims per head-group (8 heads x 64)
NPAIR = 4         # head pairs per group
NQB = 4           # q blocks of 512
NKT = 16          # key tiles of 128
KT = DM // P      # contraction tiles for projections

F32 = mybir.dt.float32
F32R = mybir.dt.float32r
BF16 = mybir.dt.bfloat16
AF = mybir.ActivationFunctionType


def _build(has_bias):
    nc = bacc.Bacc(None, target_bir_lowering=False)
    xqT = nc.dram_tensor("xqT", [DM, S], F32R, kind="ExternalInput")
    xkT = nc.dram_tensor("xkT", [DM, S], F32R, kind="ExternalInput")
    xvT = nc.dram_tensor("xvT", [DM, S], F32R, kind="ExternalInput")
    wqT = nc.dram_tensor("wqT", [DM, DG], F32R, kind="ExternalInput")
    wkT = nc.dram_tensor("wkT", [DM, DG], F32R, kind="ExternalInput")
    wvT = nc.dram_tensor("wvT", [DM, DG], F32R, kind="ExternalInput")
    woT = nc.dram_tensor("woT", [DG, DM], F32R, kind="ExternalInput")
    if has_bias:
        bq = nc.dram_tensor("bq", [1, DG], F32R, kind="ExternalInput")
        bk = nc.dram_tensor("bk", [1, DG], F32R, kind="ExternalInput")
        bv = nc.dram_tensor("bv", [1, DG], F32R, kind="ExternalInput")
    out = nc.dram_tensor("out", [S, DM], F32, kind="ExternalOutput")

    xT = {"q": xqT, "k": xkT, "v": xvT}
    wT = {"q": wqT, "k": wkT, "v": wvT}

    with TileContext(nc) as tc:
        with tc.tile_pool(name="pres", bufs=1) as pres, \
             tc.tile_pool(name="pw", bufs=3) as pw, \
             tc.tile_pool(name="px", bufs=4) as px, \
             tc.tile_pool(name="pxv", bufs=2) as pxv, \
             tc.tile_pool(name="pex", bufs=2) as pex, \
             tc.tile_pool(name="psmall", bufs=2) as psmall, \
             tc.tile_pool(name="pout", bufs=1) as pout, \
             tc.tile_pool(name="ps_proj", bufs=2, space="PSUM") as ps_proj, \
             tc.tile_pool(name="ps_sc", bufs=2, space="PSUM") as ps_sc, \
             tc.tile_pool(name="ps_av", bufs=2, space="PSUM") as ps_av:

            # resident tensors
            qT_sb = [pres.tile([P, S], BF16, name=f"qT{p}") for p in range(NPAIR)]
            kT_sb = [pres.tile([P, S], BF16, name=f"kT{p}") for p in range(NPAIR)]
            v_sb = pres.tile([P, NKT, 8, 65], BF16)
            attn_sb = pres.tile([P, NPAIR, S], F32R)
            # transposed softmax-sum collection: one tile per q-block,
            # [128 partitions, 8 (2p+h), 4 cols] so the reciprocal runs on
            # all 128 lanes (4 elems/lane) instead of 1 lane x 512
            sT = [pres.tile([P, 8, 4], F32, name=f"sT{qb}") for qb in range(NQB)]

            # weights: wq/wk/wv are dead after pair 0 and wo is only
            # needed from pair 3, so 3 rotating slots cover all four
            w_dram = {"q": wqT, "k": wkT, "v": wvT}
            w_sb = {}

            def ensure_w(key):
                if key in w_sb:
                    return
                if key == "o":
                    t = pw.tile([P, NPAIR, DM], F32R, tag="w", name="wo")
                    nc.sync.dma_start(
                        t[:], woT.rearrange("(kp p) o -> p kp o", p=P))
                else:
                    t = pw.tile([P, KT, DG], F32R, tag="w", name=f"w{key}")
                    nc.sync.dma_start(
                        t[:],
                        w_dram[key].rearrange("(kt p) n -> p kt n", p=P))
                w_sb[key] = t

            nc.vector.memset(v_sb[:, :, :, 64:65], 1.0)

            if has_bias:
                x9 = pres.tile([P, DG], F32R)      # ones row, rest zero
                xv9 = pres.tile([P, P], F32R)
                w9 = {
                    "q": pres.tile([P, DG], F32R, name="w9q"),
                    "k": pres.tile([P, DG], F32R, name="w9k"),
                    "v": pres.tile([P, DG], F32R, name="w9v"),
                }
                for t in (x9, xv9, w9["q"], w9["k"], w9["v"]):
                    nc.vector.memset(t[:], 0.0)
                nc.vector.memset(x9[0:1, :], 1.0)
                nc.vector.memset(xv9[0:1, :], 1.0)
                for key, d in (("q", bq), ("k", bk), ("v", bv)):
                    nc.sync.dma_start(w9[key][0:1, :], d[:])

            emitted = set()
            pending = []          # deferred emitters, dripped between groups
            x_tiles = {}

            def load_x(proj, nb):
                key = ("x", proj, nb)
                if key in emitted:
                    return x_tiles[(proj, nb)]
                emitted.add(key)
                xs = []
                half = (KT + 1) // 2
                for j in range(2):
                    lo = j * half
                    hi = min(KT, lo + half)
                    xt = px.tile([P, half, DG], F32R, tag="x",
                                 name=f"x_{proj}{nb}_{j}")
                    nc.sync.dma_start(
                        xt[:, 0:hi - lo, :],
                        xT[proj].rearrange("(kt p) s -> p kt s", p=P)
                        [:, lo:hi, nb * DG:(nb + 1) * DG],
                    )
                    xs.append(xt)
                x_tiles[(proj, nb)] = xs
                return xs

            def qk_subblock(proj, nb, p):
                """Project q or k for seq block nb, one pair."""
                ensure_w(proj)
                dst = qT_sb if proj == "q" else kT_sb
                xs = load_x(proj, nb)
                ps = ps_proj.tile([P, DG], F32, tag="pp",
                                  name=f"ps_{proj}{nb}_{p}")
                for kt in range(KT):
                    nc.tensor.matmul(
                        ps[:], w_sb[proj][:, kt, p * P:(p + 1) * P],
                        xs[kt // ((KT + 1) // 2)][:, kt % ((KT + 1) // 2), :],
                        start=(kt == 0), stop=(kt == KT - 1 and not has_bias),
                    )
                if has_bias:
                    nc.tensor.matmul(
                        ps[:], w9[proj][:, p * P:(p + 1) * P], x9[:],
                        start=False, stop=True,
                    )
                nc.vector.tensor_copy(dst[p][:, nb * DG:(nb + 1) * DG], ps[:])

            def v_block(m):
                """Project v for key tile m (128 positions, all 8 heads):
                fills v_sb[:, m, :, 0:64]."""
                ensure_w("v")
                xt = pxv.tile([P, KT, P], F32R, tag="xv", name=f"xv{m}")
                nc.sync.dma_start(
                    xt[:],
                    xvT.rearrange("(kt p) s -> p kt s", p=P)
                    [:, :, m * P:(m + 1) * P],
                )
                ps = ps_proj.tile([P, DG], F32, tag="pp", name=f"ps_v{m}")
                for kt in range(KT):
                    nc.tensor.matmul(
                        ps[:], xt[:, kt, :], w_sb["v"][:, kt, :],
                        start=(kt == 0), stop=(kt == KT - 1 and not has_bias),
                    )
                if has_bias:
                    nc.tensor.matmul(ps[:], xv9[:], w9["v"][:],
                                     start=False, stop=True)
                nc.vector.tensor_copy(
                    v_sb[:, m, :, 0:64],
                    ps[:].rearrange("p (h d) -> p h d", d=64),
                )

            def ensure(key):
                if key in emitted:
                    return
                emitted.add(key)
                kind = key[0]
                if kind == "v":
                    v_block(key[1])
                else:
                    kind, nb, p = key
                    qk_subblock(kind, nb, p)
                    for p2 in range(NPAIR):
                        if ("_defer", kind, nb, p2) not in emitted \
                                and (kind, nb, p2) not in emitted:
                            emitted.add(("_defer", kind, nb, p2))
                            pending.append((kind, nb, p2))

            def drip():
                while pending:
                    key = pending.pop(0)
                    if key in emitted:
                        continue
                    emitted.add(key)
                    if key[0] == "oproj":
                        oproj_subblock(key[1], key[2])
                    else:
                        qk_subblock(*key)
                    return

            def oproj_subblock(m, n):
                ensure_w("o")
                ps = ps_proj.tile([P, DG], F32, tag="pp", name=f"ps_o{m}_{n}")
                for kp in range(NPAIR):
                    nc.tensor.matmul(
                        ps[:], attn_sb[:, kp, m * P:(m + 1) * P],
                        w_sb["o"][:, kp, n * DG:(n + 1) * DG],
                        start=(kp == 0), stop=(kp == NPAIR - 1),
                    )
                ot = pout.tile([P, DG], F32, tag="ot", name=f"ot{m}_{n}")
                nc.vector.tensor_copy(ot[:], ps[:])
                nc.sync.dma_start(
                    out[m * P:(m + 1) * P, n * DG:(n + 1) * DG], ot[:])

            def oproj_chunk(qb):
                """Queue output projection for seq rows qb*512..+512."""
                for mi in range(4):
                    for n in range(2):
                        pending.append(("oproj", 4 * qb + mi, n))

            # attention: pair-outer, q-block, one key tile per group
            # (2-bank double-buffered scores tiles so exp(g) overlaps
            # scores(g+1) and the PE never sees a >1us gap)
            for p in range(NPAIR):
                for qb in range(NQB):
                    ensure(("q", qb, p))
                    av = [
                        ps_av.tile([65, DG], F32, tag="av", name=f"av{p}_{qb}_{h}")
                        for h in range(2)
                    ]
                    for g in range(NKT):
                        ensure(("k", g // 4, p))
                        ensure(("v", g))
                        drip()
                        sc = ps_sc.tile([P, 2, DG], F32, tag="sc",
                                        name=f"sc{p}_{qb}_{g}")
                        for h in range(2):
                            nc.tensor.matmul(
                                sc[:, h, :],
                                kT_sb[p][64 * h:64 * h + 64,
                                          g * P:(g + 1) * P],
                                qT_sb[p][64 * h:64 * h + 64,
                                          qb * DG:(qb + 1) * DG],
                                start=True, stop=True,
                                tile_position=(64 * h, 0),
                            )
                        ex = pex.tile([P, 2, DG], BF16, tag="ex",
                                      name=f"ex{p}_{qb}_{g}")
                        nc.scalar.activation(ex[:], sc[:], AF.Exp, scale=0.125)
                        for h in range(2):
                            nc.tensor.matmul(
                                av[h][:],
                                v_sb[:, g, 2 * p + h, :],
                                ex[:, h, :],
                                start=(g == 0),
                                stop=(g == NKT - 1),
                            )
                    for h in range(2):
                        j = 2 * p + h
                        # sums row: psum -> sbuf staging (DVE), then a
                        # transposing DMA into the 128-lane collection
                        s0 = psmall.tile([1, DG], F32, tag="s0",
                                         name=f"s0_{p}_{qb}_{h}")
                        nc.vector.tensor_scalar_mul(
                            s0[0:1, :], av[h][64:65, :], 1.0)
                        nc.sync.dma_start(
                            sT[qb][:, j, :],
                            s0[0:1, :].rearrange("o (p c) -> o p c", c=4),
                        )
                        nc.vector.tensor_copy(
                            attn_sb[64 * h:64 * h + 64, p,
                                    qb * DG:(qb + 1) * DG],
                            av[h][0:64, :],
                        )
                    if p == NPAIR - 1:
                        # batched normalization for this q-block (all pairs)
                        rqt = psmall.tile([P, 8, 4], F32, tag="rqt",
                                          name=f"rqt{qb}")
                        with nc.allow_low_precision(
                                reason="softmax denominators, fp32 recip"):
                            nc.vector.reciprocal(rqt[:], sT[qb][:])
                        for pp in range(NPAIR):
                            for h in range(2):
                                j = 2 * pp + h
                                r0 = psmall.tile([1, DG], F32, tag="r0",
                                                 name=f"r0_{qb}_{pp}_{h}")
                                nc.sync.dma_start(
                                    r0[0:1, :].rearrange("o (p c) -> o p c", c=4),
                                    rqt[:, j, :],
                                )
                                rbc = psmall.tile([P, DG], F32, tag="rbc",
                                                  name=f"rbc{qb}_{pp}_{h}")
                                nc.gpsimd.partition_broadcast(rbc[:], r0[0:1, :])
                                sl = attn_sb[64 * h:64 * h + 64, pp,
                                             qb * DG:(qb + 1) * DG]
                                nc.vector.tensor_tensor(
                                    sl, sl, rbc[64 * h:64 * h + 64, :],
                                    mybir.AluOpType.mult)
                        oproj_chunk(qb)
            while pending:
                drip()
    nc.compile()
    return nc


_CACHE = {}


def _get_nc(has_bias):
    if has_bias not in _CACHE:
        _CACHE[has_bias] = _build(has_bias)
    return _CACHE[has_bias]


def _tr(a):
    return np.ascontiguousarray(np.asarray(a, dtype=np.float32).T)


def _run(Q, K, V, Wq, bq, Wk, bk, Wv, bv, Wo, bo, trace=False):
    Q, K, V = (np.asarray(t, np.float32) for t in (Q, K, V))
    Wq, Wk, Wv, Wo = (np.asarray(t, np.float32) for t in (Wq, Wk, Wv, Wo))
    bq, bk, bv, bo = (np.asarray(t, np.float32) for t in (bq, bk, bv, bo))
    B = Q.shape[0]
    has_bias = bool(np.any(bq) or np.any(bk) or np.any(bv))
    nc = _get_nc(has_bias)

    xts = [(_tr(Q[b]), _tr(K[b]), _tr(V[b])) for b in range(B)]
    wts = []
    for g in range(2):
        sl = slice(DG * g, DG * (g + 1))
        wts.append({
            "wqT": _tr(Wq[sl]), "wkT": _tr(Wk[sl]), "wvT": _tr(Wv[sl]),
            "woT": _tr(Wo[:, sl]),
            "bq": np.ascontiguousarray(bq[None, sl]),
            "bk": np.ascontiguousarray(bk[None, sl]),
            "bv": np.ascontiguousarray(bv[None, sl]),
        })
    in_maps = []
    for c in range(8):
        b, g = c // 2, c % 2
        m = {
            "xqT": xts[b][0], "xkT": xts[b][1], "xvT": xts[b][2],
            "wqT": wts[g]["wqT"], "wkT": wts[g]["wkT"],
            "wvT": wts[g]["wvT"], "woT": wts[g]["woT"],
        }
        if has_bias:
            m["bq"] = wts[g]["bq"]
            m["bk"] = wts[g]["bk"]
            m["bv"] = wts[g]["bv"]
        in_maps.append(m)

    res = run_bass_kernel_spmd(nc, in_maps, core_ids=list(range(8)),
                               trace=trace)
    outp = np.empty((B, S, DM), np.float32)
    for b in range(B):
        outp[b] = res.results[2 * b]["out"] + res.results[2 * b + 1]["out"]
    outp += bo[None, None, :]
    return outp, res


def kernel(Q, K, V, Wq, bq, Wk, bk, Wv, bv, Wo, bo):
    outp, _ = _run(Q, K, V, Wq, bq, Wk, bk, Wv, bv, Wo, bo, trace=False)
    return outp


# revision 16
# speedup vs baseline: 1.0110x; 1.0110x over previous
"""Multi-head attention (B=4, S=2048, d_model=1024, h=16) on 8 TRN2 NeuronCores.

Sharding: data-parallel over batch (4) x tensor-parallel over head-groups (2 x 8
heads, column-split Wq/Wk/Wv, row-split Wo). Each core computes a full (2048,
1024) partial of the output projection for its (batch, head-group); the host
sums the two group partials per batch and adds bo.

Device kernel (identical SPMD program on all 8 cores):
  qT/kT = W @ X.T computed directly in head-major layout (TF32 matmuls, PE at
  full rate), scoresT = k @ qT per head with 64x128 row-tiled matmul pairs (two
  heads run concurrently on the two halves of the PE array), one 2048-wide exp
  per 4-bank PSUM block on the scalar engine (the per-instruction overhead
  makes narrow activations 40% slower), AV as [v|1].T @ exps so the softmax
  denominators fall out of the matmul for free, normalization via vector-engine
  reciprocal + gpsimd partition-broadcast, then the output projection from the
  already-transposed attention output.
"""
import numpy as np

import concourse.bacc as bacc
import concourse.mybir as mybir
from concourse.tile import TileContext
from concourse.bass_utils import run_bass_kernel_spmd

P = 128
S = 2048          # sequence length
DM = 1024         # d_model
DG = 512          # dims per head-group (8 heads x 64)
NPAIR = 4         # head pairs per group
NQB = 4           # q blocks of 512
NKT = 16          # key tiles of 128
KT = DM // P      # contraction tiles for projections

F32 = mybir.dt.float32
F32R = mybir.dt.float32r
BF16 = mybir.dt.bfloat16
AF = mybir.ActivationFunctionType


def _build(has_bias):
    nc = bacc.Bacc(None, target_bir_lowering=False)
    xqT = nc.dram_tensor("xqT", [DM, S], F32R, kind="ExternalInput")
    xkT = nc.dram_tensor("xkT", [DM, S], F32R, kind="ExternalInput")
    xvT = nc.dram_tensor("xvT", [DM, S], F32R, kind="ExternalInput")
    wqT = nc.dram_tensor("wqT", [DM, DG], F32R, kind="ExternalInput")
    wkT = nc.dram_tensor("wkT", [DM, DG], F32R, kind="ExternalInput")
    wvT = nc.dram_tensor("wvT", [DM, DG], F32R, kind="ExternalInput")
    woT = nc.dram_tensor("woT", [DG, DM], F32R, kind="ExternalInput")
    if has_bias:
        bq = nc.dram_tensor("bq", [1, DG], F32R, kind="ExternalInput")
        bk = nc.dram_tensor("bk", [1, DG], F32R, kind="ExternalInput")
        bv = nc.dram_tensor("bv", [1, DG], F32R, kind="ExternalInput")
    out = nc.dram_tensor("out", [S, DM], F32, kind="ExternalOutput")

    xT = {"q": xqT, "k": xkT, "v": xvT}
    wT = {"q": wqT, "k": wkT, "v": wvT}

    with TileContext(nc) as tc:
        with tc.tile_pool(name="pres", bufs=1) as pres, \
             tc.tile_pool(name="pw", bufs=3) as pw, \
             tc.tile_pool(name="px", bufs=4) as px, \
             tc.tile_pool(name="pxv", bufs=2) as pxv, \
             tc.tile_pool(name="pex", bufs=2) as pex, \
             tc.tile_pool(name="psmall", bufs=2) as psmall, \
             tc.tile_pool(name="pout", bufs=1) as pout, \
             tc.tile_pool(name="ps_proj", bufs=2, space="PSUM") as ps_proj, \
             tc.tile_pool(name="ps_sc", bufs=2, space="PSUM") as ps_sc, \
             tc.tile_pool(name="ps_av", bufs=2, space="PSUM") as ps_av:

            # resident tensors
            qT_sb = [pres.tile([P, S], BF16, name=f"qT{p}") for p in range(NPAIR)]
            kT_sb = [pres.tile([P, S], BF16, name=f"kT{p}") for p in range(NPAIR)]
            v_sb = pres.tile([P, NKT, 8, 65], BF16)
            attn_sb = pres.tile([P, NPAIR, S], F32R)
            # transposed softmax-sum collection: one tile per q-block,
            # [128 partitions, 8 (2p+h), 4 cols] so the reciprocal runs on
            # all 128 lanes (4 elems/lane) instead of 1 lane x 512
            sT = [pres.tile([P, 8, 4], F32, name=f"sT{qb}") for qb in range(NQB)]

            # weights: wq/wk/wv are dead after pair 0 and wo is only
            # needed from pair 3, so 3 rotating slots cover all four
            w_dram = {"q": wqT, "k": wkT, "v": wvT}
            w_sb = {}

            def ensure_w(key):
                if key in w_sb:
                    return
                if key == "o":
                    t = pw.tile([P, NPAIR, DM], F32R, tag="w", name="wo")
                    nc.sync.dma_start(
                        t[:], woT.rearrange("(kp p) o -> p kp o", p=P))
                else:
                    t = pw.tile([P, KT, DG], F32R, tag="w", name=f"w{key}")
                    nc.sync.dma_start(
                        t[:],
                        w_dram[key].rearrange("(kt p) n -> p kt n", p=P))
                w_sb[key] = t

            nc.vector.memset(v_sb[:, :, :, 64:65], 1.0)

            if has_bias:
                x9 = pres.tile([P, DG], F32R)      # ones row, rest zero
                xv9 = pres.tile([P, P], F32R)
                w9 = {
                    "q": pres.tile([P, DG], F32R, name="w9q"),
                    "k": pres.tile([P, DG], F32R, name="w9k"),
                    "v": pres.tile([P, DG], F32R, name="w9v"),
                }
                for t in (x9, xv9, w9["q"], w9["k"], w9["v"]):
                    nc.vector.memset(t[:], 0.0)
                nc.vector.memset(x9[0:1, :], 1.0)
                nc.vector.memset(xv9[0:1, :], 1.0)
                for key, d in (("q", bq), ("k", bk), ("v", bv)):
                    nc.sync.dma_start(w9[key][0:1, :], d[:])

            emitted = set()
            pending = []          # deferred emitters, dripped between groups
            x_tiles = {}

            def load_x(proj, nb, p):
                xs = []
                half = (KT + 1) // 2
                for j in range(2):
                    lo = j * half
                    hi = min(KT, lo + half)
                    xt = px.tile([P, half, DG], F32R, tag="x",
                                 name=f"x_{proj}{nb}_{p}_{j}")
                    nc.sync.dma_start(
                        xt[:, 0:hi - lo, :],
                        xT[proj].rearrange("(kt p) s -> p kt s", p=P)
                        [:, lo:hi, nb * DG:(nb + 1) * DG],
                    )
                    xs.append(xt)
                return xs

            def qk_subblock(proj, nb, p):
                """Project q or k for seq block nb, one pair."""
                ensure_w(proj)
                dst = qT_sb if proj == "q" else kT_sb
                xs = load_x(proj, nb, p)
                ps = ps_proj.tile([P, DG], F32, tag="pp",
                                  name=f"ps_{proj}{nb}_{p}")
                for kt in range(KT):
                    nc.tensor.matmul(
                        ps[:], w_sb[proj][:, kt, p * P:(p + 1) * P],
                        xs[kt // ((KT + 1) // 2)][:, kt % ((KT + 1) // 2), :],
                        start=(kt == 0), stop=(kt == KT - 1 and not has_bias),
                    )
                if has_bias:
                    nc.tensor.matmul(
                        ps[:], w9[proj][:, p * P:(p + 1) * P], x9[:],
                        start=False, stop=True,
                    )
                nc.vector.tensor_copy(dst[p][:, nb * DG:(nb + 1) * DG], ps[:])

            def v_block(m):
                """Project v for key tile m (128 positions, all 8 heads):
                fills v_sb[:, m, :, 0:64]."""
                ensure_w("v")
                xt = pxv.tile([P, KT, P], F32R, tag="xv", name=f"xv{m}")
                nc.sync.dma_start(
                    xt[:],
                    xvT.rearrange("(kt p) s -> p kt s", p=P)
                    [:, :, m * P:(m + 1) * P],
                )
                ps = ps_proj.tile([P, DG], F32, tag="pp", name=f"ps_v{m}")
                for kt in range(KT):
                    nc.tensor.matmul(
                        ps[:], xt[:, kt, :], w_sb["v"][:, kt, :],
                        start=(kt == 0), stop=(kt == KT - 1 and not has_bias),
                    )
                if has_bias:
                    nc.tensor.matmul(ps[:], xv9[:], w9["v"][:],
                                     start=False, stop=True)
                nc.vector.tensor_copy(
                    v_sb[:, m, :, 0:64],
                    ps[:].rearrange("p (h d) -> p h d", d=64),
                )

            def ensure(key):
                if key in emitted:
                    return
                emitted.add(key)
                kind = key[0]
                if kind == "v":
                    v_block(key[1])
                else:
                    qk_subblock(*key)

            queued = set()

            def queue(key):
                if key not in emitted and key not in queued:
                    queued.add(key)
                    pending.append(key)

            def drip():
                while pending:
                    key = pending.pop(0)
                    if key in emitted:
                        continue
                    emitted.add(key)
                    if key[0] == "oproj":
                        oproj_subblock(key[1], key[2])
                    else:
                        qk_subblock(*key)
                    return

            def oproj_subblock(m, n):
                ensure_w("o")
                ps = ps_proj.tile([P, DG], F32, tag="pp", name=f"ps_o{m}_{n}")
                for kp in range(NPAIR):
                    nc.tensor.matmul(
                        ps[:], attn_sb[:, kp, m * P:(m + 1) * P],
                        w_sb["o"][:, kp, n * DG:(n + 1) * DG],
                        start=(kp == 0), stop=(kp == NPAIR - 1),
                    )
                ot = pout.tile([P, DG], F32, tag="ot", name=f"ot{m}_{n}")
                nc.vector.tensor_copy(ot[:], ps[:])
                nc.sync.dma_start(
                    out[m * P:(m + 1) * P, n * DG:(n + 1) * DG], ot[:])

            def oproj_chunk(qb):
                """Queue output projection for seq rows qb*512..+512."""
                for mi in range(4):
                    for n in range(2):
                        pending.append(("oproj", 4 * qb + mi, n))

            # attention: pair-outer, q-block, one key tile per group
            # (2-bank double-buffered scores tiles so exp(g) overlaps
            # scores(g+1) and the PE never sees a >1us gap)
            for p in range(NPAIR):
                for qb in range(NQB):
                    ensure(("q", qb, p))
                    if p + 1 < NPAIR:
                        queue(("q", qb, p + 1))
                    av = [
                        ps_av.tile([65, DG], F32, tag="av", name=f"av{p}_{qb}_{h}")
                        for h in range(2)
                    ]
                    for g in range(NKT):
                        ensure(("k", g // 4, p))
                        if g % 4 == 0 and p + 1 < NPAIR:
                            queue(("k", g // 4, p + 1))
                        ensure(("v", g))
                        drip()
                        sc = ps_sc.tile([P, 2, DG], F32, tag="sc",
                                        name=f"sc{p}_{qb}_{g}")
                        for h in range(2):
                            nc.tensor.matmul(
                                sc[:, h, :],
                                kT_sb[p][64 * h:64 * h + 64,
                                          g * P:(g + 1) * P],
                                qT_sb[p][64 * h:64 * h + 64,
                                          qb * DG:(qb + 1) * DG],
                                start=True, stop=True,
                                tile_position=(64 * h, 0),
                            )
                        ex = pex.tile([P, 2, DG], BF16, tag="ex",
                                      name=f"ex{p}_{qb}_{g}")
                        nc.scalar.activation(ex[:], sc[:], AF.Exp, scale=0.125)
                        for h in range(2):
                            nc.tensor.matmul(
                                av[h][:],
                                v_sb[:, g, 2 * p + h, :],
                                ex[:, h, :],
                                start=(g == 0),
                                stop=(g == NKT - 1),
                            )
                    for h in range(2):
                        j = 2 * p + h
                        # sums row: psum -> sbuf staging (DVE), then a
                        # transposing DMA into the 128-lane collection
                        s0 = psmall.tile([1, DG], F32, tag="s0",
                                         name=f"s0_{p}_{qb}_{h}")
                        nc.vector.tensor_scalar_mul(
                            s0[0:1, :], av[h][64:65, :], 1.0)
                        nc.sync.dma_start(
                            sT[qb][:, j, :],
                            s0[0:1, :].rearrange("o (p c) -> o p c", c=4),
                        )
                        nc.vector.tensor_copy(
                            attn_sb[64 * h:64 * h + 64, p,
                                    qb * DG:(qb + 1) * DG],
                            av[h][0:64, :],
                        )
                    if p == NPAIR - 1:
                        # batched normalization for this q-block (all pairs)
                        rqt = psmall.tile([P, 8, 4], F32, tag="rqt",
                                          name=f"rqt{qb}")
                        with nc.allow_low_precision(
                                reason="softmax denominators, fp32 recip"):
                            nc.vector.reciprocal(rqt[:], sT[qb][:])
                        for pp in range(NPAIR):
                            for h in range(2):
                                j = 2 * pp + h
                                r0 = psmall.tile([1, DG], F32, tag="r0",
                                                 name=f"r0_{qb}_{pp}_{h}")
                                nc.sync.dma_start(
                                    r0[0:1, :].rearrange("o (p c) -> o p c", c=4),
                                    rqt[:, j, :],
                                )
                                rbc = psmall.tile([P, DG], F32, tag="rbc",
                                                  name=f"rbc{qb}_{pp}_{h}")
                                nc.gpsimd.partition_broadcast(rbc[:], r0[0:1, :])
                                sl = attn_sb[64 * h:64 * h + 64, pp,
                                             qb * DG:(qb + 1) * DG]
                                nc.vector.tensor_tensor(
                                    sl, sl, rbc[64 * h:64 * h + 64, :],
                                    mybir.AluOpType.mult)
                        oproj_chunk(qb)
            while pending:
                drip()
    nc.compile()
    return nc


_CACHE = {}


def _get_nc(has_bias):
    if has_bias not in _CACHE:
        _CACHE[has_bias] = _build(has_bias)
    return _CACHE[has_bias]


def _tr(a):
    return np.ascontiguousarray(np.asarray(a, dtype=np.float32).T)


def _run(Q, K, V, Wq, bq, Wk, bk, Wv, bv, Wo, bo, trace=False):
    Q, K, V = (np.asarray(t, np.float32) for t in (Q, K, V))
    Wq, Wk, Wv, Wo = (np.asarray(t, np.float32) for t in (Wq, Wk, Wv, Wo))
    bq, bk, bv, bo = (np.asarray(t, np.float32) for t in (bq, bk, bv, bo))
    B = Q.shape[0]
    has_bias = bool(np.any(bq) or np.any(bk) or np.any(bv))
    nc = _get_nc(has_bias)

    xts = [(_tr(Q[b]), _tr(K[b]), _tr(V[b])) for b in range(B)]
    wts = []
    for g in range(2):
        sl = slice(DG * g, DG * (g + 1))
        wts.append({
            "wqT": _tr(Wq[sl]), "wkT": _tr(Wk[sl]), "wvT": _tr(Wv[sl]),
            "woT": _tr(Wo[:, sl]),
            "bq": np.ascontiguousarray(bq[None, sl]),
            "bk": np.ascontiguousarray(bk[None, sl]),
            "bv": np.ascontiguousarray(bv[None, sl]),
        })
    in_maps = []
    for c in range(8):
        b, g = c // 2, c % 2
        m = {
            "xqT": xts[b][0], "xkT": xts[b][1], "xvT": xts[b][2],
            "wqT": wts[g]["wqT"], "wkT": wts[g]["wkT"],
            "wvT": wts[g]["wvT"], "woT": wts[g]["woT"],
        }
        if has_bias:
            m["bq"] = wts[g]["bq"]
            m["bk"] = wts[g]["bk"]
            m["bv"] = wts[g]["bv"]
        in_maps.append(m)

    res = run_bass_kernel_spmd(nc, in_maps, core_ids=list(range(8)),
                               trace=trace)
    outp = np.empty((B, S, DM), np.float32)
    for b in range(B):
        outp[b] = res.results[2 * b]["out"] + res.results[2 * b + 1]["out"]
    outp += bo[None, None, :]
    return outp, res


def kernel(Q, K, V, Wq, bq, Wk, bk, Wv, bv, Wo, bo):
    outp, _ = _run(Q, K, V, Wq, bq, Wk, bk, Wv, bv, Wo, bo, trace=False)
    return outp


# revision 19
# speedup vs baseline: 1.0689x; 1.0573x over previous
"""Multi-head attention (B=4, S=2048, d_model=1024, h=16) on 8 TRN2 NeuronCores.

Sharding: data-parallel over batch (4) x tensor-parallel over head-groups (2 x 8
heads, column-split Wq/Wk/Wv, row-split Wo). Each core computes a full (2048,
1024) partial of the output projection for its (batch, head-group); the host
sums the two group partials per batch and adds bo.

Device kernel (identical SPMD program on all 8 cores):
  qT/kT = W @ X.T computed directly in head-major layout (TF32 matmuls at full
  PE rate), scoresT = k @ qT per head with 64x128 row-tiled matmul pairs (two
  heads run concurrently on the two halves of the PE array), one 1024-wide exp
  per double-buffered 2-bank PSUM scores block on the scalar engine, AV as
  [v|1].T @ exps so the softmax denominators fall out of the matmul for free,
  normalization via a 128-lane reciprocal on DMA-transposed sums + gpsimd
  partition-broadcast, then the output projection from the already-transposed
  attention output. Projection sub-blocks and output-projection blocks are
  dripped one per attention group to fill the PE under the ACT-bound exp
  stream.
"""
import numpy as np

import concourse.bacc as bacc
import concourse.mybir as mybir
from concourse.tile import TileContext
from concourse.bass_utils import run_bass_kernel_spmd

P = 128
S = 2048          # sequence length
DM = 1024         # d_model
DG = 512          # dims per head-group (8 heads x 64)
NPAIR = 4         # head pairs per group
NQB = 4           # q blocks of 512
NKT = 16          # key tiles of 128
KT = DM // P      # contraction tiles for projections

F32 = mybir.dt.float32
F32R = mybir.dt.float32r
BF16 = mybir.dt.bfloat16
AF = mybir.ActivationFunctionType


def _build(has_bias):
    nc = bacc.Bacc(None, target_bir_lowering=False)
    xqT = nc.dram_tensor("xqT", [DM, S], F32R, kind="ExternalInput")
    xkT = nc.dram_tensor("xkT", [DM, S], F32R, kind="ExternalInput")
    xvT = nc.dram_tensor("xvT", [DM, S], F32R, kind="ExternalInput")
    wqT = nc.dram_tensor("wqT", [DM, DG], F32R, kind="ExternalInput")
    wkT = nc.dram_tensor("wkT", [DM, DG], F32R, kind="ExternalInput")
    wvT = nc.dram_tensor("wvT", [DM, DG], F32R, kind="ExternalInput")
    woT = nc.dram_tensor("woT", [DG, DM], F32R, kind="ExternalInput")
    if has_bias:
        bq = nc.dram_tensor("bq", [1, DG], F32R, kind="ExternalInput")
        bk = nc.dram_tensor("bk", [1, DG], F32R, kind="ExternalInput")
        bv = nc.dram_tensor("bv", [1, DG], F32R, kind="ExternalInput")
    out = nc.dram_tensor("out", [S, DM], F32, kind="ExternalOutput")

    xT = {"q": xqT, "k": xkT, "v": xvT}

    with TileContext(nc) as tc:
        with tc.tile_pool(name="pres", bufs=1) as pres, \
             tc.tile_pool(name="pw", bufs=3) as pw, \
             tc.tile_pool(name="px", bufs=4) as px, \
             tc.tile_pool(name="pxv", bufs=2) as pxv, \
             tc.tile_pool(name="pex", bufs=2) as pex, \
             tc.tile_pool(name="psmall", bufs=2) as psmall, \
             tc.tile_pool(name="pout", bufs=1) as pout, \
             tc.tile_pool(name="ps_proj", bufs=2, space="PSUM") as ps_proj, \
             tc.tile_pool(name="ps_sc", bufs=2, space="PSUM") as ps_sc, \
             tc.tile_pool(name="ps_av", bufs=2, space="PSUM") as ps_av:

            # resident tensors
            qT_sb = [pres.tile([P, S], BF16, name=f"qT{p}")
                     for p in range(NPAIR)]
            kT_sb = [pres.tile([P, S], BF16, name=f"kT{p}")
                     for p in range(NPAIR)]
            v_sb = pres.tile([P, NKT, 8, 65], BF16)
            attn_sb = pres.tile([P, NPAIR, S], F32R)
            # transposed softmax-sum collection: one tile per q-block,
            # [128 partitions, 8 (2p+h), 4 cols] so the reciprocal runs on
            # all 128 lanes (4 elems/lane) instead of 1 lane x 512
            sT = [pres.tile([P, 8, 4], F32, name=f"sT{qb}")
                  for qb in range(NQB)]

            # weights: wq/wk/wv are dead after pair 0 and wo is only
            # needed from pair 3, so 3 rotating slots cover all four
            w_dram = {"q": wqT, "k": wkT, "v": wvT}
            w_sb = {}

            def ensure_w(key):
                if key in w_sb:
                    return
                if key == "o":
                    t = pw.tile([P, NPAIR, DM], F32R, tag="w", name="wo")
                    nc.sync.dma_start(
                        t[:], woT.rearrange("(kp p) o -> p kp o", p=P))
                else:
                    t = pw.tile([P, KT, DG], F32R, tag="w", name=f"w{key}")
                    nc.sync.dma_start(
                        t[:],
                        w_dram[key].rearrange("(kt p) n -> p kt n", p=P))
                w_sb[key] = t

            nc.vector.memset(v_sb[:, :, :, 64:65], 1.0)

            if has_bias:
                x9 = pres.tile([P, DG], F32R)      # ones row, rest zero
                xv9 = pres.tile([P, P], F32R)
                w9 = {
                    "q": pres.tile([P, DG], F32R, name="w9q"),
                    "k": pres.tile([P, DG], F32R, name="w9k"),
                    "v": pres.tile([P, DG], F32R, name="w9v"),
                }
                for t in (x9, xv9, w9["q"], w9["k"], w9["v"]):
                    nc.vector.memset(t[:], 0.0)
                nc.vector.memset(x9[0:1, :], 1.0)
                nc.vector.memset(xv9[0:1, :], 1.0)
                for key, d in (("q", bq), ("k", bk), ("v", bv)):
                    nc.sync.dma_start(w9[key][0:1, :], d[:])

            emitted = set()
            queued = set()
            pending = []          # deferred emitters, dripped between groups
            x_tiles = {}

            def load_x(proj, nb, p):
                # k x-tiles are shared across the 4 pair sub-blocks (the
                # siblings drip out within a few groups); q reloads per
                # pair so its DMA spreads over the whole kernel
                shared = proj == "k"
                key = ("x", proj, nb) if shared else ("x", proj, nb, p)
                if key in x_tiles:
                    return x_tiles[key]
                xs = []
                half = (KT + 1) // 2
                for j in range(2):
                    lo = j * half
                    hi = min(KT, lo + half)
                    xt = px.tile([P, half, DG], F32R, tag="x",
                                 name=f"x_{proj}{nb}_{p}_{j}")
                    nc.sync.dma_start(
                        xt[:, 0:hi - lo, :],
                        xT[proj].rearrange("(kt p) s -> p kt s", p=P)
                        [:, lo:hi, nb * DG:(nb + 1) * DG],
                    )
                    xs.append(xt)
                x_tiles[key] = xs
                return xs

            def qk_subblock(proj, nb, p):
                """Project q or k for seq block nb, one pair."""
                ensure_w(proj)
                dst = qT_sb if proj == "q" else kT_sb
                xs = load_x(proj, nb, p)
                half = (KT + 1) // 2
                ps = ps_proj.tile([P, DG], F32, tag="pp",
                                  name=f"ps_{proj}{nb}_{p}")
                for kt in range(KT):
                    nc.tensor.matmul(
                        ps[:], w_sb[proj][:, kt, p * P:(p + 1) * P],
                        xs[kt // half][:, kt % half, :],
                        start=(kt == 0),
                        stop=(kt == KT - 1 and not has_bias),
                    )
                if has_bias:
                    nc.tensor.matmul(
                        ps[:], w9[proj][:, p * P:(p + 1) * P], x9[:],
                        start=False, stop=True,
                    )
                nc.vector.tensor_copy(dst[p][:, nb * DG:(nb + 1) * DG], ps[:])

            def v_block(m):
                """Project v for key tile m (128 positions, all 8 heads)."""
                ensure_w("v")
                xt = pxv.tile([P, KT, P], F32R, tag="xv", name=f"xv{m}")
                nc.sync.dma_start(
                    xt[:],
                    xvT.rearrange("(kt p) s -> p kt s", p=P)
                    [:, :, m * P:(m + 1) * P],
                )
                ps = ps_proj.tile([P, DG], F32, tag="pp", name=f"ps_v{m}")
                for kt in range(KT):
                    nc.tensor.matmul(
                        ps[:], xt[:, kt, :], w_sb["v"][:, kt, :],
                        start=(kt == 0),
                        stop=(kt == KT - 1 and not has_bias),
                    )
                if has_bias:
                    nc.tensor.matmul(ps[:], xv9[:], w9["v"][:],
                                     start=False, stop=True)
                nc.vector.tensor_copy(
                    v_sb[:, m, :, 0:64],
                    ps[:].rearrange("p (h d) -> p h d", d=64),
                )

            def oproj_subblock(m, n):
                ensure_w("o")
                ps = ps_proj.tile([P, DG], F32, tag="pp", name=f"ps_o{m}_{n}")
                for kp in range(NPAIR):
                    nc.tensor.matmul(
                        ps[:], attn_sb[:, kp, m * P:(m + 1) * P],
                        w_sb["o"][:, kp, n * DG:(n + 1) * DG],
                        start=(kp == 0), stop=(kp == NPAIR - 1),
                    )
                ot = pout.tile([P, DG], F32, tag="ot", name=f"ot{m}_{n}")
                nc.vector.tensor_copy(ot[:], ps[:])
                nc.sync.dma_start(
                    out[m * P:(m + 1) * P, n * DG:(n + 1) * DG], ot[:])

            def queue(key):
                if key not in emitted and key not in queued:
                    queued.add(key)
                    pending.append(key)

            def ensure(key):
                if key in emitted:
                    return
                emitted.add(key)
                kind = key[0]
                if kind == "v":
                    v_block(key[1])
                else:
                    qk_subblock(*key)
                    if kind == "k":
                        for p2 in range(NPAIR):
                            queue(("k", key[1], p2))

            def drip():
                while pending:
                    key = pending.pop(0)
                    if key in emitted:
                        continue
                    emitted.add(key)
                    if key[0] == "oproj":
                        oproj_subblock(key[1], key[2])
                    else:
                        qk_subblock(*key)
                    return

            def oproj_chunk(qb):
                """Queue output projection for seq rows qb*512..+512."""
                for mi in range(4):
                    for n in range(2):
                        pending.append(("oproj", 4 * qb + mi, n))

            def scores_mm(p, qb, g):
                sc = ps_sc.tile([P, 2, DG], F32, tag="sc",
                                name=f"sc{p}_{qb}_{g}")
                for h in range(2):
                    nc.tensor.matmul(
                        sc[:, h, :],
                        kT_sb[p][64 * h:64 * h + 64, g * P:(g + 1) * P],
                        qT_sb[p][64 * h:64 * h + 64, qb * DG:(qb + 1) * DG],
                        start=True, stop=True,
                        tile_position=(64 * h, 0),
                    )
                return sc

            # attention: pair-outer, q-block, one key tile per group.
            # scores(g+1) are emitted before AV(g) so the PE computes them
            # under exp(g) and the exp cadence stays at the ACT floor.
            for p in range(NPAIR):
                for qb in range(NQB):
                    ensure(("q", qb, p))
                    if qb + 1 < NQB:
                        queue(("q", qb + 1, p))
                    elif p + 1 < NPAIR:
                        queue(("q", 0, p + 1))
                    av = [
                        ps_av.tile([65, DG], F32, tag="av",
                                   name=f"av{p}_{qb}_{h}")
                        for h in range(2)
                    ]
                    ensure(("k", 0, p))
                    ensure(("v", 0))
                    sc_cur = scores_mm(p, qb, 0)
                    for g in range(NKT):
                        if g + 1 < NKT:
                            ensure(("k", (g + 1) // 4, p))
                            ensure(("v", g + 1))
                        ex = pex.tile([P, 2, DG], BF16, tag="ex",
                                      name=f"ex{p}_{qb}_{g}")
                        nc.scalar.activation(ex[:], sc_cur[:], AF.Exp,
                                             scale=0.125)
                        if g + 1 < NKT:
                            sc_cur = scores_mm(p, qb, g + 1)
                        for h in range(2):
                            nc.tensor.matmul(
                                av[h][:],
                                v_sb[:, g, 2 * p + h, :],
                                ex[:, h, :],
                                start=(g == 0),
                                stop=(g == NKT - 1),
                            )
                        drip()
                    # boundary: stage sums, evacuate unnormalized attn
                    for h in range(2):
                        j = 2 * p + h
                        s0 = psmall.tile([1, DG], F32, tag="s0",
                                         name=f"s0_{p}_{qb}_{h}")
                        nc.vector.tensor_scalar_mul(
                            s0[0:1, :], av[h][64:65, :], 1.0)
                        nc.sync.dma_start(
                            sT[qb][:, j, :],
                            s0[0:1, :].rearrange("o (p c) -> o p c", c=4),
                        )
                        nc.vector.tensor_copy(
                            attn_sb[64 * h:64 * h + 64, p,
                                    qb * DG:(qb + 1) * DG],
                            av[h][0:64, :],
                        )
                    # per-(p,qb) normalization, off the critical path:
                    # 128-lane reciprocal on the transposed sums, DMA each
                    # row back to partition 0, broadcast, multiply in place
                    rqt = psmall.tile([P, 2, 4], F32, tag="rqt",
                                      name=f"rqt{p}_{qb}")
                    with nc.allow_low_precision(
                            reason="softmax denominators, fp32 recip"):
                        nc.vector.reciprocal(
                            rqt[:], sT[qb][:, 2 * p:2 * p + 2, :])
                    for h in range(2):
                        r0 = psmall.tile([1, DG], F32, tag="r0",
                                         name=f"r0_{qb}_{p}_{h}")
                        nc.sync.dma_start(
                            r0[0:1, :].rearrange("o (p c) -> o p c", c=4),
                            rqt[:, h, :],
                        )
                        rbc = psmall.tile([P, DG], F32, tag="rbc",
                                          name=f"rbc{qb}_{p}_{h}")
                        nc.gpsimd.partition_broadcast(rbc[:], r0[0:1, :])
                        sl = attn_sb[64 * h:64 * h + 64, p,
                                     qb * DG:(qb + 1) * DG]
                        nc.vector.tensor_tensor(
                            sl, sl, rbc[64 * h:64 * h + 64, :],
                            mybir.AluOpType.mult)
                    if p == NPAIR - 1:
                        oproj_chunk(qb)
            while pending:
                drip()
    nc.compile()
    return nc


_CACHE = {}


def _get_nc(has_bias):
    if has_bias not in _CACHE:
        _CACHE[has_bias] = _build(has_bias)
    return _CACHE[has_bias]


def _tr(a):
    return np.ascontiguousarray(np.asarray(a, dtype=np.float32).T)


def _run(Q, K, V, Wq, bq, Wk, bk, Wv, bv, Wo, bo, trace=False):
    Q, K, V = (np.asarray(t, np.float32) for t in (Q, K, V))
    Wq, Wk, Wv, Wo = (np.asarray(t, np.float32) for t in (Wq, Wk, Wv, Wo))
    bq, bk, bv, bo = (np.asarray(t, np.float32) for t in (bq, bk, bv, bo))
    B = Q.shape[0]
    has_bias = bool(np.any(bq) or np.any(bk) or np.any(bv))
    nc = _get_nc(has_bias)

    xts = [(_tr(Q[b]), _tr(K[b]), _tr(V[b])) for b in range(B)]
    wts = []
    for g in range(2):
        sl = slice(DG * g, DG * (g + 1))
        wts.append({
            "wqT": _tr(Wq[sl]), "wkT": _tr(Wk[sl]), "wvT": _tr(Wv[sl]),
            "woT": _tr(Wo[:, sl]),
            "bq": np.ascontiguousarray(bq[None, sl]),
            "bk": np.ascontiguousarray(bk[None, sl]),
            "bv": np.ascontiguousarray(bv[None, sl]),
        })
    in_maps = []
    for c in range(8):
        b, g = c // 2, c % 2
        m = {
            "xqT": xts[b][0], "xkT": xts[b][1], "xvT": xts[b][2],
            "wqT": wts[g]["wqT"], "wkT": wts[g]["wkT"],
            "wvT": wts[g]["wvT"], "woT": wts[g]["woT"],
        }
        if has_bias:
            m["bq"] = wts[g]["bq"]
            m["bk"] = wts[g]["bk"]
            m["bv"] = wts[g]["bv"]
        in_maps.append(m)

    res = run_bass_kernel_spmd(nc, in_maps, core_ids=list(range(8)),
                               trace=trace)
    outp = np.empty((B, S, DM), np.float32)
    for b in range(B):
        outp[b] = res.results[2 * b]["out"] + res.results[2 * b + 1]["out"]
    outp += bo[None, None, :]
    return outp, res


def kernel(Q, K, V, Wq, bq, Wk, bk, Wv, bv, Wo, bo):
    outp, _ = _run(Q, K, V, Wq, bq, Wk, bk, Wv, bv, Wo, bo, trace=False)
    return outp
